# revision 38
# baseline (speedup 1.0000x reference)
"""Multi-head self-attention (RoPE + causal) Trainium2 Bass kernel.

Problem: b=2, s=2048, d_model=1024, 16 heads x 64 dims, causal, RoPE.
Sharding over 8 NeuronCores: core c -> (batch bi = c//4, head group g = c%4
of 4 heads). Each core computes its 4 heads' attention from x[bi] and
produces a partial output projection (Wo column-block); the host sums the
4 partials per batch element.

v2 layout (all matmul operands bf16, fp32 PSUM accumulate):
  xT      16x[128, 1024]  x[bi]^T halves (d_model on partitions)
  wqkv    8x[128, 768]    [WqT | WkT | WvT], Q/K rows permuted per head:
                          [h even dims(32), h odd dims(32)] so RoPE is a
                          32-row swap + elementwise mul/add
  QK proj lhsT = W slices (few weight loads), psum [128,1024] halves,
          RoPE applied straight out of PSUM (DVE for Q, gpsimd for K)
  scores  ST[k,q] per (pair t, kblock j, qchunk): both heads in one
          [128, 2, 512] psum; exp on ACT (scale 1/8), causal mask via
          gpsimd affine_select on the diagonal block
  AV      per-head per-qchunk single-bank accumulators [128, 512]; V row
          64 = ones column so the softmax denominator rides along
  norm    DVE reciprocal + gpsimd partition_broadcast + DVE mul (PSUM in)
  out     yT[dm, s] = Wo^T-block @ O^T, psum halves -> bf16 -> DMA; host
          transposes and sums the 4 partials per batch element.
"""

import os
import sys
from contextlib import ExitStack

import numpy as np

for _p in ("/root/.axon_site", "/root/.axon_site/_ro/trn_rl_repo", "/opt/trn_rl_repo"):
    if os.path.isdir(_p) and _p not in sys.path:
        sys.path.append(_p)

import ml_dtypes  # noqa: E402
import concourse.bass as bass  # noqa: E402
import concourse.tile as tile  # noqa: E402
import concourse.mybir as mybir  # noqa: E402
from concourse import bacc  # noqa: E402
from concourse.bass import ts  # noqa: E402
from concourse.bass_utils import run_bass_kernel_spmd  # noqa: E402

BF16 = mybir.dt.bfloat16
F32 = mybir.dt.float32
NPBF16 = ml_dtypes.bfloat16

S = 2048
D = 1024
DK = 64
THETA = 10000.0

_CACHE = {}


def _build_nc():
    nc = bacc.Bacc("TRN2", target_bir_lowering=False, debug=False, num_devices=8)
    xT = nc.dram_tensor("xT", [D, S], BF16, kind="ExternalInput").ap()
    wqkv = nc.dram_tensor("wqkv", [D, 768], BF16, kind="ExternalInput").ap()
    woT = nc.dram_tensor("woT", [256, D], BF16, kind="ExternalInput").ap()
    ropec = nc.dram_tensor("ropec", [128, 2048], BF16, kind="ExternalInput").ap()
    ropes = nc.dram_tensor("ropes", [128, 2048], BF16, kind="ExternalInput").ap()
    yp = nc.dram_tensor("yp", [D, S], BF16, kind="ExternalOutput").ap()

    Exp = mybir.ActivationFunctionType.Exp

    with ExitStack() as ctx:
        tc = ctx.enter_context(tile.TileContext(nc))
        const = ctx.enter_context(tc.tile_pool(name="const", bufs=1))
        sb = ctx.enter_context(tc.tile_pool(name="sb", bufs=2))
        expp = ctx.enter_context(tc.tile_pool(name="expp", bufs=8))
        outp = ctx.enter_context(tc.tile_pool(name="outp", bufs=3))
        ps = ctx.enter_context(tc.tile_pool(name="ps", bufs=3, space="PSUM"))
        psav = ctx.enter_context(tc.tile_pool(name="psav", bufs=1, space="PSUM"))

        # ---- persistent SBUF ----
        xts = [
            [const.tile([128, 1024], BF16, tag=f"xt{i}_{h}", name=f"xt{i}_{h}")
             for h in range(2)]
            for i in range(8)
        ]
        wts = [const.tile([128, 768], BF16, tag=f"wt{i}", name=f"wt{i}") for i in range(8)]
        wos = [const.tile([128, D], BF16, tag=f"wo{i}", name=f"wo{i}") for i in range(2)]
        ropec_sb = const.tile([128, 2048], BF16, tag="ropec")
        ropes_sb = const.tile([128, 2048], BF16, tag="ropes")
        # V per key-block: 4 heads x (64 dims + ones col)
        vt = [const.tile([128, 260], BF16, tag=f"v{j}", name=f"v{j}") for j in range(16)]
        # Q/K projected+roped halves: [t][half] -> [128, 1024]
        qf = [[const.tile([128, 1024], BF16, tag=f"qf{t}_{h}", name=f"qf{t}_{h}")
               for h in range(2)] for t in range(2)]
        kf = [[const.tile([128, 1024], BF16, tag=f"kf{t}_{h}", name=f"kf{t}_{h}")
               for h in range(2)] for t in range(2)]
        # attention output O^T per pair: rows = 2 heads x 64 dims
        ot = [const.tile([128, S], BF16, tag=f"ot{t}", name=f"ot{t}") for t in range(2)]

        # ---- input DMAs (interleaved so V-proj's kc-loop starts ASAP) ----
        for i in range(8):
            nc.sync.dma_start(wts[i][:], wqkv[ts(i, 128), :])
            nc.sync.dma_start(xts[i][0][:], xT[ts(i, 128), 0:1024])
        nc.sync.dma_start(ropec_sb[:], ropec[:])
        nc.sync.dma_start(ropes_sb[:], ropes[:])
        for i in range(8):
            nc.sync.dma_start(xts[i][1][:], xT[ts(i, 128), 1024:2048])
        for i in range(2):
            nc.sync.dma_start(wos[i][:], woT[ts(i, 128), :])

        # PE warmup: dependency-free junk matmuls execute during the input
        # DMA wait, ramping the tensor engine's p-state before real work.
        wtmp = const.tile([128, 512], BF16, tag="wtmp")
        nc.gpsimd.memset(wtmp[:], 0.0)
        for _ in range(20):
            wps = ps.tile([128, 512], F32, tag="big", name="wps")
            nc.tensor.matmul(wps[:], lhsT=wtmp[:, 0:128], rhs=wtmp[:], start=True, stop=True)

        # ones columns of vt tiles
        for j in range(16):
            ones_ap = vt[j][:].rearrange("p (h x) -> p h x", h=4)[:, :, 64:65]
            nc.gpsimd.memset(ones_ap, 1.0)

        # ones row for the PE denominator broadcast
        ones1 = const.tile([1, 64], BF16, tag="ones1")
        nc.gpsimd.memset(ones1[:], 1.0)

        # causal triangle mask tile, doubled for both heads of a pair:
        # tri2[k, hh, c] = 1 if c >= k else 0
        tri2 = const.tile([128, 2, 128], BF16, tag="tri2")
        nc.gpsimd.memset(tri2[:], 1.0)
        nc.gpsimd.affine_select(
            out=tri2[:], in_=tri2[:], compare_op=mybir.AluOpType.is_ge,
            fill=0.0, base=0, pattern=[[0, 2], [1, 128]], channel_multiplier=-1,
        )

        # ---- V projection: V[key, vdim] natural layout ----
        # One feeder step = one key-block (8 matmuls + fill). Interleaved
        # steps (mid-attention) fill via DVE to stay off the saturated ACT.
        def v_step(st, inter=False):
            def f():
                vp = ps.tile([128, 256], F32, tag="big", name="vp")
                for kc in range(8):
                    nc.tensor.matmul(
                        vp[:],
                        lhsT=xts[kc][st // 8][:, ts(st % 8, 128)],
                        rhs=wts[kc][:, 512:768],
                        start=(kc == 0),
                        stop=(kc == 7),
                    )
                dst = vt[st][:].rearrange("p (h x) -> p h x", h=4)[:, :, 0:64]
                vsrc = vp[:].rearrange("p (h x) -> p h x", h=4)
                if inter:
                    nc.vector.tensor_copy(dst, vsrc)
                else:
                    nc.scalar.copy(dst, vsrc)
            return f

        def vproj(lo, hi, inter=False):
            for st in range(lo, hi):
                v_step(st, inter)()

        # ---- Q/K projection + RoPE (as a list of feeder steps) ----
        # 8 matmul steps accumulate the projection; the tail step applies
        # RoPE: one DVE copy evicts PSUM (releasing the ring slot), the row
        # swap goes through SB->SB DMAs (partition remap is free on the DMA
        # engines), then 3 partition-aligned DVE muls.
        def qk_steps(t, qk, H, act_evict=False):
            wcol = (0 if qk == 0 else 256) + t * 128
            dstt = qf[t] if qk == 0 else kf[t]
            state = {}

            def mm_step(kc0):
                def f():
                    if kc0 == 0:
                        state["qkp"] = ps.tile(
                            [128, 1024], F32, tag="big", name="qkp"
                        )
                    qkp = state["qkp"]
                    for kc in (kc0, kc0 + 1):
                        for c in range(2):
                            nc.tensor.matmul(
                                qkp[:, ts(c, 512)],
                                lhsT=wts[kc][:, wcol : wcol + 128],
                                rhs=xts[kc][H][:, ts(c, 512)],
                                start=(kc == 0),
                                stop=(kc == 7),
                            )
                return f

            def rope_tail():
                qkp = state["qkp"]
                kb = sb.tile([128, 1024], BF16, tag=f"kb{qk}", name="kb")
                if act_evict:
                    nc.scalar.copy(kb[:], qkp[:])
                else:
                    nc.vector.tensor_copy(kb[:], qkp[:])
                kbs = sb.tile([128, 1024], BF16, tag=f"kbs{qk}", name="kbs")
                nc.sync.dma_start(kbs[0:32, :], kb[32:64, :])
                nc.sync.dma_start(kbs[32:64, :], kb[0:32, :])
                nc.sync.dma_start(kbs[64:96, :], kb[96:128, :])
                nc.sync.dma_start(kbs[96:128, :], kb[64:96, :])
                t1 = sb.tile([128, 1024], BF16, tag=f"t1{qk}", name="t1")
                nc.vector.tensor_mul(t1[:], kb[:], ropec_sb[:, ts(H, 1024)])
                t2 = sb.tile([128, 1024], BF16, tag=f"t2{qk}", name="t2")
                nc.vector.tensor_mul(t2[:], kbs[:], ropes_sb[:, ts(H, 1024)])
                nc.vector.tensor_add(dstt[H][:], t1[:], t2[:])

            return [mm_step(kc) for kc in range(0, 8, 2)] + [rope_tail]

        def qk_proj(t, qk, H, act_evict=False):
            for f in qk_steps(t, qk, H, act_evict):
                f()

        # ---- attention for pair t, single qchunk qc (512 queries) ----
        # Software-pipelined: scores for chunk j+LOOKAHEAD are emitted before
        # the AV matmuls of chunk j, so the PE never waits for ACT's exp.
        def attn(t, qc, feeders=(), per_chunk=2):
            fq = list(feeders)

            def feed(n):
                for _ in range(n):
                    if fq:
                        fq.pop(0)()

            av = [
                psav.tile([128, 512], F32, tag=f"av{hh}", name=f"av{hh}")
                for hh in range(2)
            ]
            js = list(range(4 * qc + 4))
            stage = {}

            def scores(j):
                q0 = 128 * j
                c_start = max(512 * qc, q0)
                w = 512 * (qc + 1) - c_start
                H = qc // 2
                cl = c_start - 1024 * H
                sps = ps.tile([128, 2, 512], F32, tag="big", name="sps")
                es = expp.tile([128, 2, 512], BF16, tag="es", name="es")
                for hh in range(2):
                    r0 = 64 * hh
                    nc.tensor.matmul(
                        sps[:, hh, 0:w],
                        lhsT=kf[t][j // 8][r0 : r0 + 64, ts(j % 8, 128)],
                        rhs=qf[t][H][r0 : r0 + 64, cl : cl + w],
                        start=True,
                        stop=True,
                    )
                nc.scalar.activation(es[:, :, 0:w], sps[:, :, 0:w], Exp, scale=0.125)
                if c_start == q0:
                    # diagonal block: zero es[k, c] for c < k, both heads in
                    # one DVE mul (kept off gpsimd's custom-library path)
                    nc.vector.tensor_mul(
                        es[:, :, 0:128], es[:, :, 0:128], tri2[:]
                    )
                stage[j] = (es, c_start % 512, w)

            def accum(j):
                es, lo, w = stage.pop(j)
                for hh in range(2):
                    hv = 2 * t + hh
                    nc.tensor.matmul(
                        av[hh][0:65, lo : lo + w],
                        lhsT=vt[j][:, 65 * hv : 65 * hv + 65],
                        rhs=es[:, hh, 0:w],
                        start=(j == 0),
                        stop=(j == js[-1]),
                    )

            LOOK = 3
            for i, j in enumerate(js):
                scores(j)
                if i >= LOOK:
                    accum(js[i - LOOK])
                feed(per_chunk)
            for j in js[-LOOK:]:
                accum(j)
            feed(len(fq))

            # normalize. Two quick DVE copies evict the PSUM accumulator
            # (so the next chunk's AV can start immediately). The denominator
            # reciprocal is broadcast across partitions with a tiny PE
            # ones-matmul (gpsimd partition_broadcast forces a ~7us gpsimd
            # library reload every time it alternates with tensor ops).
            # recip straight off PSUM partition 64 is broken on HW, hence
            # the cross-copy to partition 0 first.
            for hh in range(2):
                a = av[hh]
                oasb = sb.tile([64, 512], BF16, tag="oasb", name="oasb")
                nc.vector.tensor_copy(oasb[:], a[0:64, :])
                dn = sb.tile([1, 512], F32, tag="dn", name="dn")
                nc.vector.tensor_copy(dn[:], a[64:65, :])
                rr = sb.tile([1, 512], F32, tag="rr", name="rr")
                nc.vector.reciprocal_approx_fast(rr[:], dn[:])
                rb = sb.tile([64, 512], F32, tag="rb", name="rb")
                nc.gpsimd.partition_broadcast(rb[:], rr[:])
                nc.vector.tensor_mul(
                    ot[t][64 * hh : 64 * hh + 64, ts(qc, 512)],
                    oasb[:],
                    rb[:],
                )

        # ---- emission schedule ----
        # Minimal pre-attention phase, then later projections / V blocks
        # are fed one step (~0.5us of PE work) at a time between attention
        # chunks, so the PE's slack under the ACT-bound exp stream absorbs
        # them without ever starving ACT.
        qk_proj(0, 1, 0, act_evict=True)  # K pair 0 half 0
        qk_proj(0, 0, 0)                  # Q pair 0 half 0
        vproj(0, 8)
        vproj(8, 16)
        qk_proj(0, 1, 1)  # K pair 0 half 1 (rope during A00/A01)
        qk_proj(0, 0, 1)
        attn(0, 0)
        attn(0, 1)
        qk_proj(1, 1, 0)  # K pair 1 half 0 (rope during A02/A03)
        qk_proj(1, 0, 0)
        attn(0, 2)
        attn(0, 3)
        qk_proj(1, 1, 1)  # K pair 1 half 1 (rope during A10/A11)
        qk_proj(1, 0, 1)
        attn(1, 0)
        attn(1, 1)
        attn(1, 2)
        attn(1, 3)

        # ---- output projection: yT[dm, s] partial over this core's 256 dims ----
        for dm in range(8):
            yt = outp.tile([128, 2048], BF16, tag="yt", name="yt")
            for H in range(2):
                pp = ps.tile([128, 1024], F32, tag="big", name="pp")
                for cc in range(2):
                    for c in range(2):
                        nc.tensor.matmul(
                            pp[:, ts(c, 512)],
                            lhsT=wos[cc][:, ts(dm, 128)],
                            rhs=ot[cc][:, 1024 * H + 512 * c : 1024 * H + 512 * (c + 1)],
                            start=(cc == 0),
                            stop=(cc == 1),
                        )
                # both ACT and DVE are idle by the output phase; alternate
                if (dm + H) % 2 == 0:
                    nc.scalar.copy(yt[:, ts(H, 1024)], pp[:])
                else:
                    nc.vector.tensor_copy(yt[:, ts(H, 1024)], pp[:])
            nc.sync.dma_start(yp[ts(dm, 128), :], yt[:])

    nc.compile()
    return nc


def _host_inputs(x, token_positions, Wq, Wk, Wv, Wo):
    x = np.asarray(x, dtype=np.float32)
    Wq = np.asarray(Wq, dtype=np.float32)
    Wk = np.asarray(Wk, dtype=np.float32)
    Wv = np.asarray(Wv, dtype=np.float32)
    Wo = np.asarray(Wo, dtype=np.float32)
    pos = np.asarray(token_positions).astype(np.float32)

    # rope tables, rows = [x1(32) x2(32)] x2 heads, freq index p%32
    f = np.arange(32, dtype=np.float32)
    inv = 1.0 / (THETA ** (2.0 * f / DK))
    ang = pos[:, None] * inv[None, :]  # [S, 32]
    cosT = np.cos(ang).T.astype(np.float32)  # [32, S]
    sinT = np.sin(ang).T.astype(np.float32)
    crow = np.tile(cosT, (4, 1))
    srow = np.concatenate([-sinT, sinT, -sinT, sinT], axis=0)

    ropec = np.ascontiguousarray(crow).astype(NPBF16)
    ropes = np.ascontiguousarray(srow).astype(NPBF16)

    ev = np.arange(0, DK, 2)
    od = np.arange(1, DK, 2)
    in_maps = []
    for core in range(8):
        bi, g = core // 4, core % 4
        xTb = np.ascontiguousarray(x[bi].T).astype(NPBF16)
        qk_idx = []
        for t in range(2):
            for hh, sel in ((2 * t, ev), (2 * t, od), (2 * t + 1, ev), (2 * t + 1, od)):
                qk_idx.append(DK * (4 * g + hh) + sel)
        qk_idx = np.concatenate(qk_idx)
        v_idx = 256 * g + np.arange(256)
        wq = Wq[qk_idx, :].T
        wk = Wk[qk_idx, :].T
        wv = Wv[v_idx, :].T
        wqkv = np.ascontiguousarray(
            np.concatenate([wq, wk, wv], axis=1)
        ).astype(NPBF16)
        woTl = np.ascontiguousarray(Wo[:, v_idx].T).astype(NPBF16)
        in_maps.append(
            dict(xT=xTb, wqkv=wqkv, woT=woTl, ropec=ropec, ropes=ropes)
        )
    return in_maps


def _run(inputs, trace=False, tmpdir=None):
    if "nc" not in _CACHE:
        _CACHE["nc"] = _build_nc()
    nc = _CACHE["nc"]
    in_maps = _host_inputs(**inputs)
    kw = {}
    if trace:
        kw = dict(trace=True, tmpdir=tmpdir)
    res = run_bass_kernel_spmd(nc, in_maps, list(range(8)), **kw)
    out = np.zeros((2, S, D), np.float32)
    for core in range(8):
        out[core // 4] += res.results[core]["yp"].astype(np.float32).T
    return out, res


def kernel(**inputs):
    out, _ = _run(inputs, trace=False)
    return out


# revision 39
# speedup vs baseline: 1.0147x; 1.0147x over previous
"""Multi-head self-attention (RoPE + causal) Trainium2 Bass kernel.

Problem: b=2, s=2048, d_model=1024, 16 heads x 64 dims, causal, RoPE.
Sharding over 8 NeuronCores: core c -> (batch bi = c//4, head group g = c%4
of 4 heads). Each core computes its 4 heads' attention from x[bi] and
produces a partial output projection (Wo column-block); the host sums the
4 partials per batch element.

v2 layout (all matmul operands bf16, fp32 PSUM accumulate):
  xT      16x[128, 1024]  x[bi]^T halves (d_model on partitions)
  wqkv    8x[128, 768]    [WqT | WkT | WvT], Q/K rows permuted per head:
                          [h even dims(32), h odd dims(32)] so RoPE is a
                          32-row swap + elementwise mul/add
  QK proj lhsT = W slices (few weight loads), psum [128,1024] halves,
          RoPE applied straight out of PSUM (DVE for Q, gpsimd for K)
  scores  ST[k,q] per (pair t, kblock j, qchunk): both heads in one
          [128, 2, 512] psum; exp on ACT (scale 1/8), causal mask via
          gpsimd affine_select on the diagonal block
  AV      per-head per-qchunk single-bank accumulators [128, 512]; V row
          64 = ones column so the softmax denominator rides along
  norm    DVE reciprocal + gpsimd partition_broadcast + DVE mul (PSUM in)
  out     yT[dm, s] = Wo^T-block @ O^T, psum halves -> bf16 -> DMA; host
          transposes and sums the 4 partials per batch element.
"""

import os
import sys
from contextlib import ExitStack

import numpy as np

for _p in ("/root/.axon_site", "/root/.axon_site/_ro/trn_rl_repo", "/opt/trn_rl_repo"):
    if os.path.isdir(_p) and _p not in sys.path:
        sys.path.append(_p)

import ml_dtypes  # noqa: E402
import concourse.bass as bass  # noqa: E402
import concourse.tile as tile  # noqa: E402
import concourse.mybir as mybir  # noqa: E402
from concourse import bacc  # noqa: E402
from concourse.bass import ts  # noqa: E402
from concourse.bass_utils import run_bass_kernel_spmd  # noqa: E402

BF16 = mybir.dt.bfloat16
F32 = mybir.dt.float32
NPBF16 = ml_dtypes.bfloat16

S = 2048
D = 1024
DK = 64
THETA = 10000.0

_CACHE = {}


def _build_nc():
    nc = bacc.Bacc("TRN2", target_bir_lowering=False, debug=False, num_devices=8)
    xT = nc.dram_tensor("xT", [D, S], BF16, kind="ExternalInput").ap()
    wqkv = nc.dram_tensor("wqkv", [D, 768], BF16, kind="ExternalInput").ap()
    woT = nc.dram_tensor("woT", [256, D], BF16, kind="ExternalInput").ap()
    ropec = nc.dram_tensor("ropec", [128, 2048], BF16, kind="ExternalInput").ap()
    ropes = nc.dram_tensor("ropes", [128, 2048], BF16, kind="ExternalInput").ap()
    yp = nc.dram_tensor("yp", [D, S], BF16, kind="ExternalOutput").ap()

    Exp = mybir.ActivationFunctionType.Exp

    with ExitStack() as ctx:
        tc = ctx.enter_context(tile.TileContext(nc))
        const = ctx.enter_context(tc.tile_pool(name="const", bufs=1))
        sb = ctx.enter_context(tc.tile_pool(name="sb", bufs=2))
        expp = ctx.enter_context(tc.tile_pool(name="expp", bufs=8))
        outp = ctx.enter_context(tc.tile_pool(name="outp", bufs=3))
        ps = ctx.enter_context(tc.tile_pool(name="ps", bufs=3, space="PSUM"))
        psav = ctx.enter_context(tc.tile_pool(name="psav", bufs=1, space="PSUM"))

        # ---- persistent SBUF ----
        xts = [
            [const.tile([128, 1024], BF16, tag=f"xt{i}_{h}", name=f"xt{i}_{h}")
             for h in range(2)]
            for i in range(8)
        ]
        wts = [const.tile([128, 768], BF16, tag=f"wt{i}", name=f"wt{i}") for i in range(8)]
        wos = [const.tile([128, D], BF16, tag=f"wo{i}", name=f"wo{i}") for i in range(2)]
        ropec_sb = const.tile([128, 2048], BF16, tag="ropec")
        ropes_sb = const.tile([128, 2048], BF16, tag="ropes")
        # V per key-block: 4 heads x (64 dims + ones col)
        vt = [const.tile([128, 260], BF16, tag=f"v{j}", name=f"v{j}") for j in range(16)]
        # Q/K projected+roped halves: [t][half] -> [128, 1024]
        qf = [[const.tile([128, 1024], BF16, tag=f"qf{t}_{h}", name=f"qf{t}_{h}")
               for h in range(2)] for t in range(2)]
        kf = [[const.tile([128, 1024], BF16, tag=f"kf{t}_{h}", name=f"kf{t}_{h}")
               for h in range(2)] for t in range(2)]
        # attention output O^T per pair: rows = 2 heads x 64 dims
        ot = [const.tile([128, S], BF16, tag=f"ot{t}", name=f"ot{t}") for t in range(2)]

        # ---- input DMAs (interleaved so V-proj's kc-loop starts ASAP) ----
        for i in range(8):
            nc.sync.dma_start(wts[i][:], wqkv[ts(i, 128), :])
            nc.sync.dma_start(xts[i][0][:], xT[ts(i, 128), 0:1024])
        nc.sync.dma_start(ropec_sb[:], ropec[:])
        nc.sync.dma_start(ropes_sb[:], ropes[:])
        for i in range(8):
            nc.sync.dma_start(xts[i][1][:], xT[ts(i, 128), 1024:2048])
        for i in range(2):
            nc.sync.dma_start(wos[i][:], woT[ts(i, 128), :])

        # PE warmup: dependency-free junk matmuls execute during the input
        # DMA wait, ramping the tensor engine's p-state before real work.
        wtmp = const.tile([128, 512], BF16, tag="wtmp")
        nc.gpsimd.memset(wtmp[:], 0.0)
        for _ in range(20):
            wps = ps.tile([128, 512], F32, tag="big", name="wps")
            nc.tensor.matmul(wps[:], lhsT=wtmp[:, 0:128], rhs=wtmp[:], start=True, stop=True)

        # ones columns of vt tiles
        for j in range(16):
            ones_ap = vt[j][:].rearrange("p (h x) -> p h x", h=4)[:, :, 64:65]
            nc.gpsimd.memset(ones_ap, 1.0)

        # ones row for the PE denominator broadcast
        ones1 = const.tile([1, 64], BF16, tag="ones1")
        nc.gpsimd.memset(ones1[:], 1.0)

        # causal triangle mask tile, doubled for both heads of a pair:
        # tri2[k, hh, c] = 1 if c >= k else 0
        tri2 = const.tile([128, 2, 128], BF16, tag="tri2")
        nc.gpsimd.memset(tri2[:], 1.0)
        nc.gpsimd.affine_select(
            out=tri2[:], in_=tri2[:], compare_op=mybir.AluOpType.is_ge,
            fill=0.0, base=0, pattern=[[0, 2], [1, 128]], channel_multiplier=-1,
        )

        # ---- V projection: V[key, vdim] natural layout ----
        # One feeder step = one key-block (8 matmuls + fill). Interleaved
        # steps (mid-attention) fill via DVE to stay off the saturated ACT.
        def v_step(st, inter=False):
            def f():
                vp = ps.tile([128, 256], F32, tag="big", name="vp")
                for kc in range(8):
                    nc.tensor.matmul(
                        vp[:],
                        lhsT=xts[kc][st // 8][:, ts(st % 8, 128)],
                        rhs=wts[kc][:, 512:768],
                        start=(kc == 0),
                        stop=(kc == 7),
                    )
                dst = vt[st][:].rearrange("p (h x) -> p h x", h=4)[:, :, 0:64]
                vsrc = vp[:].rearrange("p (h x) -> p h x", h=4)
                if inter:
                    nc.vector.tensor_copy(dst, vsrc)
                else:
                    nc.scalar.copy(dst, vsrc)
            return f

        def vproj(lo, hi, inter=False):
            for st in range(lo, hi):
                v_step(st, inter)()

        # ---- Q/K projection + RoPE (as a list of feeder steps) ----
        # 8 matmul steps accumulate the projection; the tail step applies
        # RoPE: one DVE copy evicts PSUM (releasing the ring slot), the row
        # swap goes through SB->SB DMAs (partition remap is free on the DMA
        # engines), then 3 partition-aligned DVE muls.
        def qk_steps(t, qk, H, act_evict=False):
            wcol = (0 if qk == 0 else 256) + t * 128
            dstt = qf[t] if qk == 0 else kf[t]
            state = {}

            def mm_step(kc0):
                def f():
                    if kc0 == 0:
                        state["qkp"] = ps.tile(
                            [128, 1024], F32, tag="big", name="qkp"
                        )
                    qkp = state["qkp"]
                    for kc in (kc0, kc0 + 1):
                        for c in range(2):
                            nc.tensor.matmul(
                                qkp[:, ts(c, 512)],
                                lhsT=wts[kc][:, wcol : wcol + 128],
                                rhs=xts[kc][H][:, ts(c, 512)],
                                start=(kc == 0),
                                stop=(kc == 7),
                            )
                return f

            def rope_tail():
                qkp = state["qkp"]
                kb = sb.tile([128, 1024], BF16, tag=f"kb{qk}", name="kb")
                if act_evict:
                    nc.scalar.copy(kb[:], qkp[:])
                else:
                    nc.vector.tensor_copy(kb[:], qkp[:])
                kbs = sb.tile([128, 1024], BF16, tag=f"kbs{qk}", name="kbs")
                nc.sync.dma_start(kbs[0:32, :], kb[32:64, :])
                nc.sync.dma_start(kbs[32:64, :], kb[0:32, :])
                nc.sync.dma_start(kbs[64:96, :], kb[96:128, :])
                nc.sync.dma_start(kbs[96:128, :], kb[64:96, :])
                t1 = sb.tile([128, 1024], BF16, tag=f"t1{qk}", name="t1")
                nc.vector.tensor_mul(t1[:], kb[:], ropec_sb[:, ts(H, 1024)])
                t2 = sb.tile([128, 1024], BF16, tag=f"t2{qk}", name="t2")
                nc.vector.tensor_mul(t2[:], kbs[:], ropes_sb[:, ts(H, 1024)])
                nc.vector.tensor_add(dstt[H][:], t1[:], t2[:])

            return [mm_step(kc) for kc in range(0, 8, 2)] + [rope_tail]

        def qk_proj(t, qk, H, act_evict=False):
            for f in qk_steps(t, qk, H, act_evict):
                f()

        # ---- attention for pair t, single qchunk qc (512 queries) ----
        # Software-pipelined: scores for chunk j+LOOKAHEAD are emitted before
        # the AV matmuls of chunk j, so the PE never waits for ACT's exp.
        def attn(t, qc, feeders=(), per_chunk=2):
            fq = list(feeders)

            def feed(n):
                for _ in range(n):
                    if fq:
                        fq.pop(0)()

            av = [
                psav.tile([128, 512], F32, tag=f"av{hh}", name=f"av{hh}")
                for hh in range(2)
            ]
            js = list(range(4 * qc + 4))
            stage = {}

            def scores(j):
                q0 = 128 * j
                c_start = max(512 * qc, q0)
                w = 512 * (qc + 1) - c_start
                H = qc // 2
                cl = c_start - 1024 * H
                sps = ps.tile([128, 2, 512], F32, tag="big", name="sps")
                es = expp.tile([128, 2, 512], BF16, tag="es", name="es")
                for hh in range(2):
                    r0 = 64 * hh
                    nc.tensor.matmul(
                        sps[:, hh, 0:w],
                        lhsT=kf[t][j // 8][r0 : r0 + 64, ts(j % 8, 128)],
                        rhs=qf[t][H][r0 : r0 + 64, cl : cl + w],
                        start=True,
                        stop=True,
                    )
                nc.scalar.activation(es[:, :, 0:w], sps[:, :, 0:w], Exp, scale=0.125)
                if c_start == q0:
                    # diagonal block: zero es[k, c] for c < k, both heads in
                    # one DVE mul (kept off gpsimd's custom-library path)
                    nc.vector.tensor_mul(
                        es[:, :, 0:128], es[:, :, 0:128], tri2[:]
                    )
                stage[j] = (es, c_start % 512, w)

            def accum(j):
                es, lo, w = stage.pop(j)
                for hh in range(2):
                    hv = 2 * t + hh
                    nc.tensor.matmul(
                        av[hh][0:65, lo : lo + w],
                        lhsT=vt[j][:, 65 * hv : 65 * hv + 65],
                        rhs=es[:, hh, 0:w],
                        start=(j == 0),
                        stop=(j == js[-1]),
                    )

            LOOK = 2
            for i, j in enumerate(js):
                scores(j)
                if i >= LOOK:
                    accum(js[i - LOOK])
                feed(per_chunk)
            for j in js[-LOOK:]:
                accum(j)
            feed(len(fq))

            # normalize. Two quick DVE copies evict the PSUM accumulator
            # (so the next chunk's AV can start immediately). The denominator
            # reciprocal is broadcast across partitions with a tiny PE
            # ones-matmul (gpsimd partition_broadcast forces a ~7us gpsimd
            # library reload every time it alternates with tensor ops).
            # recip straight off PSUM partition 64 is broken on HW, hence
            # the cross-copy to partition 0 first.
            for hh in range(2):
                a = av[hh]
                oasb = sb.tile([64, 512], BF16, tag="oasb", name="oasb")
                nc.vector.tensor_copy(oasb[:], a[0:64, :])
                dn = sb.tile([1, 512], F32, tag="dn", name="dn")
                nc.vector.tensor_copy(dn[:], a[64:65, :])
                rr = sb.tile([1, 512], F32, tag="rr", name="rr")
                nc.vector.reciprocal_approx_fast(rr[:], dn[:])
                rb = sb.tile([64, 512], F32, tag="rb", name="rb")
                nc.gpsimd.partition_broadcast(rb[:], rr[:])
                nc.vector.tensor_mul(
                    ot[t][64 * hh : 64 * hh + 64, ts(qc, 512)],
                    oasb[:],
                    rb[:],
                )

        # ---- emission schedule ----
        # Minimal pre-attention phase, then later projections / V blocks
        # are fed one step (~0.5us of PE work) at a time between attention
        # chunks, so the PE's slack under the ACT-bound exp stream absorbs
        # them without ever starving ACT.
        qk_proj(0, 1, 0, act_evict=True)  # K pair 0 half 0
        qk_proj(0, 0, 0)                  # Q pair 0 half 0
        vproj(0, 8)
        vproj(8, 16)
        qk_proj(0, 1, 1)  # K pair 0 half 1 (rope during A00/A01)
        qk_proj(0, 0, 1)
        attn(0, 0)
        attn(0, 1)
        qk_proj(1, 1, 0)  # K pair 1 half 0 (rope during A02/A03)
        qk_proj(1, 0, 0)
        attn(0, 2)
        attn(0, 3)
        qk_proj(1, 1, 1)  # K pair 1 half 1 (rope during A10/A11)
        qk_proj(1, 0, 1)
        attn(1, 0)
        attn(1, 1)
        attn(1, 2)
        attn(1, 3)

        # ---- output projection: yT[dm, s] partial over this core's 256 dims ----
        for dm in range(8):
            yt = outp.tile([128, 2048], BF16, tag="yt", name="yt")
            for H in range(2):
                pp = ps.tile([128, 1024], F32, tag="big", name="pp")
                for cc in range(2):
                    for c in range(2):
                        nc.tensor.matmul(
                            pp[:, ts(c, 512)],
                            lhsT=wos[cc][:, ts(dm, 128)],
                            rhs=ot[cc][:, 1024 * H + 512 * c : 1024 * H + 512 * (c + 1)],
                            start=(cc == 0),
                            stop=(cc == 1),
                        )
                # both ACT and DVE are idle by the output phase; alternate
                if (dm + H) % 2 == 0:
                    nc.scalar.copy(yt[:, ts(H, 1024)], pp[:])
                else:
                    nc.vector.tensor_copy(yt[:, ts(H, 1024)], pp[:])
            nc.sync.dma_start(yp[ts(dm, 128), :], yt[:])

    nc.compile()
    return nc


def _host_inputs(x, token_positions, Wq, Wk, Wv, Wo):
    x = np.asarray(x, dtype=np.float32)
    Wq = np.asarray(Wq, dtype=np.float32)
    Wk = np.asarray(Wk, dtype=np.float32)
    Wv = np.asarray(Wv, dtype=np.float32)
    Wo = np.asarray(Wo, dtype=np.float32)
    pos = np.asarray(token_positions).astype(np.float32)

    # rope tables, rows = [x1(32) x2(32)] x2 heads, freq index p%32
    f = np.arange(32, dtype=np.float32)
    inv = 1.0 / (THETA ** (2.0 * f / DK))
    ang = pos[:, None] * inv[None, :]  # [S, 32]
    cosT = np.cos(ang).T.astype(np.float32)  # [32, S]
    sinT = np.sin(ang).T.astype(np.float32)
    crow = np.tile(cosT, (4, 1))
    srow = np.concatenate([-sinT, sinT, -sinT, sinT], axis=0)

    ropec = np.ascontiguousarray(crow).astype(NPBF16)
    ropes = np.ascontiguousarray(srow).astype(NPBF16)

    ev = np.arange(0, DK, 2)
    od = np.arange(1, DK, 2)
    in_maps = []
    for core in range(8):
        bi, g = core // 4, core % 4
        xTb = np.ascontiguousarray(x[bi].T).astype(NPBF16)
        qk_idx = []
        for t in range(2):
            for hh, sel in ((2 * t, ev), (2 * t, od), (2 * t + 1, ev), (2 * t + 1, od)):
                qk_idx.append(DK * (4 * g + hh) + sel)
        qk_idx = np.concatenate(qk_idx)
        v_idx = 256 * g + np.arange(256)
        wq = Wq[qk_idx, :].T
        wk = Wk[qk_idx, :].T
        wv = Wv[v_idx, :].T
        wqkv = np.ascontiguousarray(
            np.concatenate([wq, wk, wv], axis=1)
        ).astype(NPBF16)
        woTl = np.ascontiguousarray(Wo[:, v_idx].T).astype(NPBF16)
        in_maps.append(
            dict(xT=xTb, wqkv=wqkv, woT=woTl, ropec=ropec, ropes=ropes)
        )
    return in_maps


def _run(inputs, trace=False, tmpdir=None):
    if "nc" not in _CACHE:
        _CACHE["nc"] = _build_nc()
    nc = _CACHE["nc"]
    in_maps = _host_inputs(**inputs)
    kw = {}
    if trace:
        kw = dict(trace=True, tmpdir=tmpdir)
    res = run_bass_kernel_spmd(nc, in_maps, list(range(8)), **kw)
    out = np.zeros((2, S, D), np.float32)
    for core in range(8):
        out[core // 4] += res.results[core]["yp"].astype(np.float32).T
    return out, res


def kernel(**inputs):
    out, _ = _run(inputs, trace=False)
    return out
